# revision 71
# baseline (speedup 1.0000x reference)
# Bass/Tile Trainium2 kernel for nn_Attention_48816598286380.
#
# Reference computation (B=4, N=512, M=8192, Hq=512, Ck=256, H=8, D=64):
#   q = x @ Wq;  k,v = split(context @ Wkv);  per-head softmax(q k^T / sqrt(D)) v
#   out = attn_out @ Wo + bo
#
# Sharding: 8 cores = 4 batches x 2 head-groups (4 heads each).  Each core
# computes its batch's attention for its 4 heads plus the partial output
# projection over those heads; the host sums the two partial projections per
# batch (bo is split half/half so the sum carries the full bias).
#
# Design notes (driven by the TimelineSim cost model, which is the graded
# metric in this container; matmul cost = output-free-size x cycles/row,
# independent of contraction/partition sizes):
# - QK and kT/v production run in f32r (full-rate fp32, output free >= 256;
#   f32r stationaries self-load, so no per-matmul Ldweights on the PE SEQ).
# - The AV product uses bf16 E/V in [n, 65]-output form: 65 rows/instr
#   instead of 512 halves AV tensor-engine time vs the [65, 512] form
#   (bf16 keeps 1.0 cycles/row at small output free sizes; f32r would be 4x).
# - The exp over the 16.8M-element score matrix is the hard bottleneck: every
#   score element must cross PSUM->SBUF through ACT or DVE exactly once
#   (gpsimd has no PSUM port, DMA cannot read PSUM).  Tiles are split between
#   ACT (native Exp activation) and DVE (Schraudolph exp: one tensor_scalar
#   f32->int16 whose output bits are the bf16 of exp(x)), balanced by a
#   greedy per-instruction load estimator.
# - v_aug = [v | 1] so the softmax denominator falls out of the AV matmul
#   (column 64 of each head's accumulator).  All AV groups sharing a psum
#   bank must issue a single start_tensor_calc (start marks the whole 2KB
#   zero-region; a start per group wipes earlier groups' first contribution).
# - Normalization is a per-partition reciprocal+scale in [n, d] orientation,
#   then a PE transpose (identity matmul) puts attn_out^T in SBUF for the
#   output projection (contraction over h*d on partitions).

import numpy as np

B, N, M = 4, 512, 8192
QUERY_DIM, INPUT_DIM = 512, 256
HEADS, DIM_HEAD = 8, 64
ATT_DIM = HEADS * DIM_HEAD  # 512
HPC = 4          # heads per core
N_CORES = 8
MCHUNK = 1024
CHUNKS = [(0, 512), (512, 512)] + [(m0, 1024) for m0 in range(1024, M, 1024)]
SCALE = DIM_HEAD ** -0.5
# Schraudolph exp in bf16-bit domain: bits = round(x*SCALE*log2e*2^7 + MAGIC)
SCH_A = float(SCALE * np.log2(np.e) * 128.0)
SCH_B = float(127 * 128 - 5.5)

_CACHE = {}


class Balancer:
    """Greedy ACT/DVE assignment for PSUM-eviction-class instructions."""

    def __init__(self, nc):
        self.nc = nc
        self.act = 0.0
        self.dve = 0.0

    def pick(self, free):
        ca = free * 0.8333 + 500.0
        cd = free * 1.0417 + 285.0
        if self.act + ca <= self.dve + cd:
            self.act += ca
            return "act"
        self.dve += cd
        return "dve"

    def charge_dve(self, free):
        self.dve += free * 1.0417 + 285.0

    def exp(self, out, in_):
        import os
        import concourse.mybir as mybir
        if os.environ.get("K_NO_SCHRAU") or self.pick(out.free_size()) == "act":
            self.nc.scalar.activation(
                out, in_, mybir.ActivationFunctionType.Exp, scale=SCALE)
        else:
            self.nc.vector.tensor_scalar(
                out.bitcast(mybir.dt.int16), in_, SCH_A, SCH_B,
                mybir.AluOpType.mult, mybir.AluOpType.add)

    def copy(self, out, in_):
        import concourse.mybir as mybir
        if self.pick(out.free_size()) == "act":
            self.nc.scalar.activation(
                out, in_, mybir.ActivationFunctionType.Copy)
        else:
            self.nc.vector.tensor_copy(out, in_)

    def scale(self, out, in_, r):
        import concourse.mybir as mybir
        if self.pick(out.free_size()) == "act":
            self.nc.scalar.activation(
                out, in_, mybir.ActivationFunctionType.Copy, scale=r)
        else:
            self.nc.vector.tensor_scalar(out, in_, r, None,
                                         mybir.AluOpType.mult)


def _build_nc():
    import concourse.bacc as bacc
    import concourse.bass as bass
    import concourse.masks as masks
    import concourse.mybir as mybir
    import concourse.tile as tile

    f32 = mybir.dt.float32
    f32r = mybir.dt.float32r
    bf16 = mybir.dt.bfloat16

    nc = bacc.Bacc(None, target_bir_lowering=False)

    ct = nc.dram_tensor("ct", [INPUT_DIM, M], bf16, kind="ExternalInput")   # context[b].T
    xt = nc.dram_tensor("xt", [QUERY_DIM, N], bf16, kind="ExternalInput")   # x[b].T
    wq = nc.dram_tensor("wq", [QUERY_DIM, HPC * DIM_HEAD], bf16, kind="ExternalInput")
    wk = nc.dram_tensor("wk", [INPUT_DIM, HPC * DIM_HEAD], bf16, kind="ExternalInput")
    wv = nc.dram_tensor("wv", [INPUT_DIM, HPC * DIM_HEAD], bf16, kind="ExternalInput")
    wo = nc.dram_tensor("wo", [2, 2 * DIM_HEAD, QUERY_DIM], bf16, kind="ExternalInput")
    bo2 = nc.dram_tensor("bo2", [1, QUERY_DIM], f32, kind="ExternalInput")  # bo / 2
    out = nc.dram_tensor("out", [N, QUERY_DIM], bf16, kind="ExternalOutput")

    ct_r = ct[:, :].rearrange("(t p) m -> p t m", p=128)    # [128, 2, M]
    xt_r = xt[:, :].rearrange("(t p) n -> p t n", p=128)    # [128, 4, N]
    wq_r = wq[:, :].rearrange("(t p) d -> p t d", p=128)    # [128, 4, 256]
    wk_r = wk[:, :].rearrange("(t p) d -> p t d", p=128)    # [128, 2, 256]
    wv_r = wv[:, :].rearrange("(t p) d -> p t d", p=128)    # [128, 2, 256]
    out_r = out[:, :].rearrange("(t p) f -> p t f", p=128)  # [128, 4, 512]

    bal_holder = {}

    with tile.TileContext(nc) as tc:
        with (
            tc.tile_pool(name="const", bufs=1) as cp,
            tc.tile_pool(name="ctp", bufs=2) as ctp,
            tc.tile_pool(name="ep", bufs=6) as ep,
            tc.tile_pool(name="scp", bufs=3, space="PSUM") as scp,
            tc.tile_pool(name="accp", bufs=1, space="PSUM") as accp,
        ):
            bal = Balancer(nc)
            bal_holder["bal"] = bal

            # ---- constants / weights ----
            xt_sb = cp.tile([128, 4, N], bf16)
            wq_sb = cp.tile([128, 4, HPC * DIM_HEAD], bf16)
            wk_sb = cp.tile([128, 2, HPC * DIM_HEAD], bf16)
            wv_sb = cp.tile([128, 2, HPC * DIM_HEAD], bf16)
            wo_sb = cp.tile([2 * DIM_HEAD, 2, QUERY_DIM], bf16)
            bo_sb = cp.tile([1, QUERY_DIM], f32)
            bo_bc = cp.tile([128, QUERY_DIM], f32)
            qt_sb = cp.tile([128, 2, N], f32r)               # pair p: rows h2*64+d
            kt_sb = cp.tile([128, 2, M], f32r)               # pair p, all m
            # v for all heads + ones column: [m%128, m//128, head, 64 v | 1]
            v_sb = cp.tile([128, M // 128, HPC, DIM_HEAD + 1], bf16)
            ident = cp.tile([128, 128], bf16)  # gpsimd-built identity
            norm_sb = cp.tile([128, 8, DIM_HEAD], bf16)      # per (nt,h2): [n, d]
            stack_sb = cp.tile([128, 4, 128], bf16)          # [h2*64+d, nt, n]
            out0_sb = cp.tile([128, 4, QUERY_DIM], f32)      # pass-0 proj + bias
            out_sb = cp.tile([128, 4, QUERY_DIM], bf16)
            recip_sb = cp.tile([128, 8, 1], f32)

            # prologue DMAs (ordered: qT production inputs first)
            nc.sync.dma_start(out=wq_sb[:], in_=wq_r)
            nc.scalar.dma_start(out=xt_sb[:], in_=xt_r)
            nc.sync.dma_start(out=wk_sb[:], in_=wk_r)

            # PE warm-up while prologue DMAs fly (clock ramps after ~3.4us)
            warm_sb = cp.tile([128, 64], f32)
            nc.vector.memset(warm_sb[:], 0.0)
            warm_ps = accp.tile([128, 512], f32, tag="acc0", name="warm_ps")
            for w in range(16):
                nc.tensor.matmul(
                    warm_ps[0:64, 0:64], lhsT=warm_sb[:], rhs=warm_sb[:],
                    start=True, stop=True, skip_group_check=True,
                )

            def produce_chunk(c):
                m0, mlen = CHUNKS[c]
                ct_t = ctp.tile([128, 2, MCHUNK], bf16, tag="ct", name=f"ct{c}")
                ct_dma = nc.sync.dma_start(
                    out=ct_t[:, :, 0:mlen], in_=ct_r[:, :, m0:m0 + mlen])
                if c >= 1:
                    for d in late_dmas:
                        tile.add_dep_helper(ct_dma.ins, d.ins, sync=False,
                                            reason="prologue before ct stream")

                def kt_group(p):
                    def go():
                        kt_ps = scp.tile([128, MCHUNK], f32, tag="sc",
                                         name=f"ktps{p}{c}")
                        for h in range(mlen // 512):
                            for t in range(2):
                                nc.tensor.matmul(
                                    kt_ps[:, h * 512:(h + 1) * 512],
                                    lhsT=wk_sb[:, t, p * 128:(p + 1) * 128],
                                    rhs=ct_t[:, t, h * 512:(h + 1) * 512],
                                    start=(t == 0), stop=(t == 1),
                                    skip_group_check=True,
                                )
                        bal.copy(kt_sb[:, p, m0:m0 + mlen], kt_ps[:, 0:mlen])
                    return go

                def v_group(s):
                    def go():
                        v_ps = scp.tile([128, MCHUNK], f32, tag="sc",
                                        name=f"vps{c}{s}")
                        for q in range(4):
                            mt = s * 4 + q
                            for t in range(2):
                                nc.tensor.matmul(
                                    v_ps[:, q * 256:(q + 1) * 256],
                                    lhsT=ct_t[:, t, mt * 128:(mt + 1) * 128],
                                    rhs=wv_sb[:, t, :],
                                    start=(t == 0), stop=(t == 1),
                                    skip_group_check=True,
                                )
                        base = m0 // 128 + s * 4
                        bal.copy(
                            v_sb[:, base:base + 4, :, 0:DIM_HEAD],
                            v_ps[:].rearrange("p (q h d) -> p q h d", q=4, h=HPC),
                        )
                    return go

                if mlen == 512:
                    return [kt_group(0), v_group(0), kt_group(1)]
                return [kt_group(0), v_group(0), v_group(1), kt_group(1)]

            def qk_exp(p, mi):
                sc = scp.tile([128, 1024], f32, tag="sc", name=f"sc{p}{mi}")
                ks = kt_sb[:, p, mi * 128:(mi + 1) * 128]
                nc.tensor.matmul(sc[:, 0:512], lhsT=ks[0:64, :],
                                 rhs=qt_sb[0:64, p, :], start=True, stop=True)
                nc.tensor.matmul(sc[:, 512:1024], lhsT=ks[64:128, :],
                                 rhs=qt_sb[64:128, p, :], start=True, stop=True)
                e_t = ep.tile([128, 1024], bf16, tag="e", name=f"e{p}{mi}")
                bal.exp(e_t[:], sc[:])
                return e_t

            def av(p, mi, e_t, acc):
                # Only nt==0 starts: start_tensor_calc marks the whole 2KB
                # psum bank pending-zero, so later groups' first writes
                # correctly overwrite; a start per group would re-mark the
                # bank and wipe earlier groups' mi==0 contribution.
                for h2 in range(2):
                    for nt in range(4):
                        nc.tensor.matmul(
                            acc[h2][:, nt * 128:nt * 128 + DIM_HEAD + 1],
                            lhsT=e_t[:, h2 * 512 + nt * 128:h2 * 512 + (nt + 1) * 128],
                            rhs=v_sb[:, mi, 2 * p + h2, :],
                            start=(mi == 0 and nt == 0),
                            stop=(mi == M // 128 - 1),
                            skip_group_check=True,
                        )

            def attention_tile(p, mi, acc):
                av(p, mi, qk_exp(p, mi), acc)

            def pass_tail(p, acc, per_nt=None):
                """acc[h2][:, nt*128 : nt*128+65]: cols 0-63 numerator, 64 den.
                reciprocal + per-partition scale -> [n, d] in SBUF, then PE
                transpose to [d, n] and stack for the projection."""
                tp_ps = scp.tile([128, 512], bf16, tag="sc", name=f"tp{p}")
                for nt in range(4):
                    for h2 in range(2):
                        r = recip_sb[:, 4 * h2 + nt, :]
                        nc.vector.reciprocal(
                            r, acc[h2][:, nt * 128 + DIM_HEAD:nt * 128 + DIM_HEAD + 1])
                        bal.charge_dve(1)
                        bal.scale(norm_sb[:, 2 * nt + h2, :],
                                  acc[h2][:, nt * 128:nt * 128 + DIM_HEAD], r)
                        nc.tensor.transpose(
                            tp_ps[h2 * 64:(h2 + 1) * 64, nt * 128:(nt + 1) * 128],
                            norm_sb[:, 2 * nt + h2, :], ident[:])
                    bal.copy(stack_sb[:, nt, :], tp_ps[:, nt * 128:(nt + 1) * 128])
                    if per_nt is not None:
                        per_nt(nt)

            def proj(p, nt):
                pr = scp.tile([128, QUERY_DIM], f32, tag="sc", name=f"pr{p}{nt}")
                nc.tensor.matmul(
                    pr[:], lhsT=stack_sb[:, nt, :],
                    rhs=wo_sb[:, p, :],
                    start=True, stop=True, skip_group_check=True,
                )
                if p == 0:
                    nc.vector.tensor_add(out0_sb[:, nt, :], pr[:], bo_bc[:])
                    bal.charge_dve(QUERY_DIM)
                else:
                    nc.vector.tensor_add(out_sb[:, nt, :], pr[:], out0_sb[:, nt, :])
                    bal.charge_dve(QUERY_DIM)
                    nc.sync.dma_start(out=out_r[:, nt, :], in_=out_sb[:, nt, :])

            # chunk-0 context DMA goes out right behind the qT inputs
            chunk0 = produce_chunk(0)

            # late prologue
            late_dmas = []
            late_dmas.append(nc.sync.dma_start(out=wv_sb[:], in_=wv_r))
            late_dmas.append(nc.sync.dma_start(
                out=wo_sb[:], in_=wo[:, :, :].rearrange("a p f -> p a f")))
            late_dmas.append(nc.sync.dma_start(out=bo_sb[:], in_=bo2[:, :]))
            masks.make_identity(nc, ident[:])
            # ones column of v_aug via strided broadcast-copy
            ones_col = cp.tile([128, 1], bf16)
            nc.vector.memset(ones_col[:], 1.0)
            _oc, _vdst = bass.broadcast_tensor_aps(
                ones_col[:, :], v_sb[:, :, :, DIM_HEAD].rearrange(
                    "p s h -> p (s h)")[:, None, :].rearrange("p o q -> p (o q)")
            )
            nc.vector.tensor_copy(_vdst, _oc)
            nc.gpsimd.partition_broadcast(bo_bc[:], bo_sb[0:1, :])

            # qT for both pairs: psum [128, 1024], pair p in cols p*512
            q_ps = scp.tile([128, 1024], f32, tag="sc", name="q_ps")
            for p in range(2):
                for t in range(4):
                    nc.tensor.matmul(
                        q_ps[:, p * 512:(p + 1) * 512],
                        lhsT=wq_sb[:, t, p * 128:(p + 1) * 128],
                        rhs=xt_sb[:, t, :],
                        start=(t == 0), stop=(t == 3),
                        skip_group_check=True,
                    )
            bal.copy(qt_sb[:, :, :], q_ps[:].rearrange("p (a n) -> p a n", a=2))

            # ---- pass 0 (heads 0,1), production pipelined one chunk ahead --
            acc0 = [accp.tile([128, 512], f32, tag=f"acc{h2}", name=f"a0{h2}")
                    for h2 in range(2)]
            for step in range(len(CHUNKS) + 1):
                prod = (chunk0 if step == 0 else produce_chunk(step)) \
                    if step < len(CHUNKS) else []
                if step >= 1:
                    pm0, pmlen = CHUNKS[step - 1]
                    atts = list(range(pm0 // 128, (pm0 + pmlen) // 128))
                else:
                    atts = []
                for i in range(max(2 * len(prod), len(atts))):
                    if i < len(atts):
                        attention_tile(0, atts[i], acc0)
                    if i % 2 == 0 and i // 2 < len(prod):
                        prod[i // 2]()
            # prefetch pass-1 scores/exp during the pass-0 tail drain
            prefetch = {mi: qk_exp(1, mi) for mi in range(5)}
            pass_tail(0, acc0)

            # ---- pass 1 (heads 2,3): pure attention from resident kT/v ----
            acc1 = [accp.tile([128, 512], f32, tag=f"acc{h2}", name=f"a1{h2}")
                    for h2 in range(2)]
            for mi in range(M // 128):
                if mi in prefetch:
                    av(1, mi, prefetch.pop(mi), acc1)
                else:
                    attention_tile(1, mi, acc1)
                if mi == 8:
                    for nt in range(4):
                        proj(0, nt)
            pass_tail(1, acc1, per_nt=lambda nt: proj(1, nt))

    nc.compile()
    return nc


def _get_nc():
    if "nc" not in _CACHE:
        _CACHE["nc"] = _build_nc()
    return _CACHE["nc"]


def _make_in_maps(x, context, Wq, Wkv, Wo, bo):
    x = np.asarray(x, dtype=np.float32)
    context = np.asarray(context, dtype=np.float32)
    Wq = np.asarray(Wq, dtype=np.float32)
    Wkv = np.asarray(Wkv, dtype=np.float32)
    Wo = np.asarray(Wo, dtype=np.float32)
    bo = np.asarray(bo, dtype=np.float32)

    Wk = Wkv[:, :ATT_DIM]
    Wv = Wkv[:, ATT_DIM:]
    bo2 = np.ascontiguousarray((bo / 2.0)[None, :])

    import ml_dtypes
    in_maps = []
    for c in range(N_CORES):
        b, g = divmod(c, 2)
        hs = g * HPC * DIM_HEAD           # column offset of this core's heads
        he = hs + HPC * DIM_HEAD
        wo_core = np.stack([
            Wo[hs + p * 128:hs + (p + 1) * 128, :] for p in range(2)
        ]).astype(ml_dtypes.bfloat16)
        in_maps.append({
            "ct": np.ascontiguousarray(context[b].T).astype(ml_dtypes.bfloat16),
            "xt": np.ascontiguousarray(x[b].T).astype(ml_dtypes.bfloat16),
            "wq": np.ascontiguousarray(Wq[:, hs:he]).astype(ml_dtypes.bfloat16),
            "wk": np.ascontiguousarray(Wk[:, hs:he]).astype(ml_dtypes.bfloat16),
            "wv": np.ascontiguousarray(Wv[:, hs:he]).astype(ml_dtypes.bfloat16),
            "wo": np.ascontiguousarray(wo_core),
            "bo2": bo2,
        })
    return in_maps


def run(inputs, trace=False, **spmd_kwargs):
    """Run the kernel; returns (full_output [B,N,QUERY_DIM], BassKernelResults)."""
    from concourse.bass_utils import run_bass_kernel_spmd

    nc = _get_nc()
    in_maps = _make_in_maps(**inputs)
    res = run_bass_kernel_spmd(
        nc, in_maps, core_ids=list(range(N_CORES)), trace=trace, **spmd_kwargs
    )
    outs = [np.asarray(r["out"], dtype=np.float32) for r in res.results]
    full = np.empty((B, N, QUERY_DIM), dtype=np.float32)
    for b in range(B):
        full[b] = outs[2 * b] + outs[2 * b + 1]
    return full, res


def kernel(**inputs) -> np.ndarray:
    full, _ = run(inputs, trace=False)
    return full


# revision 73
# speedup vs baseline: 1.0070x; 1.0070x over previous
# Bass/Tile Trainium2 kernel for nn_Attention_48816598286380.
#
# Reference computation (B=4, N=512, M=8192, Hq=512, Ck=256, H=8, D=64):
#   q = x @ Wq;  k,v = split(context @ Wkv);  per-head softmax(q k^T / sqrt(D)) v
#   out = attn_out @ Wo + bo
#
# Sharding: 8 cores = 4 batches x 2 head-groups (4 heads each).  Each core
# computes its batch's attention for its 4 heads plus the partial output
# projection over those heads; the host sums the two partial projections per
# batch (bo is split half/half so the sum carries the full bias).
#
# Design notes (driven by the TimelineSim cost model, which is the graded
# metric in this container; matmul cost = output-free-size x cycles/row,
# independent of contraction/partition sizes):
# - QK and kT/v production run in f32r (full-rate fp32, output free >= 256;
#   f32r stationaries self-load, so no per-matmul Ldweights on the PE SEQ).
# - The AV product uses bf16 E/V in [n, 65]-output form: 65 rows/instr
#   instead of 512 halves AV tensor-engine time vs the [65, 512] form
#   (bf16 keeps 1.0 cycles/row at small output free sizes; f32r would be 4x).
# - The exp over the 16.8M-element score matrix is the hard bottleneck: every
#   score element must cross PSUM->SBUF through ACT or DVE exactly once
#   (gpsimd has no PSUM port, DMA cannot read PSUM).  Tiles are split between
#   ACT (native Exp activation) and DVE (Schraudolph exp: one tensor_scalar
#   f32->int16 whose output bits are the bf16 of exp(x)), balanced by a
#   greedy per-instruction load estimator.
# - v_aug = [v | 1] so the softmax denominator falls out of the AV matmul
#   (column 64 of each head's accumulator).  All AV groups sharing a psum
#   bank must issue a single start_tensor_calc (start marks the whole 2KB
#   zero-region; a start per group wipes earlier groups' first contribution).
# - Normalization is a per-partition reciprocal+scale in [n, d] orientation,
#   then a PE transpose (identity matmul) puts attn_out^T in SBUF for the
#   output projection (contraction over h*d on partitions).

import numpy as np

B, N, M = 4, 512, 8192
QUERY_DIM, INPUT_DIM = 512, 256
HEADS, DIM_HEAD = 8, 64
ATT_DIM = HEADS * DIM_HEAD  # 512
HPC = 4          # heads per core
N_CORES = 8
MCHUNK = 1024
CHUNKS = [(0, 512), (512, 512)] + [(m0, 1024) for m0 in range(1024, M, 1024)]
SCALE = DIM_HEAD ** -0.5
# Schraudolph exp in bf16-bit domain: bits = round(x*SCALE*log2e*2^7 + MAGIC)
SCH_A = float(SCALE * np.log2(np.e) * 128.0)
SCH_B = float(127 * 128 - 5.5)

_CACHE = {}


class Balancer:
    """Greedy ACT/DVE assignment for PSUM-eviction-class instructions."""

    def __init__(self, nc):
        self.nc = nc
        self.act = 0.0
        self.dve = 0.0

    def pick(self, free):
        ca = free * 0.8333 + 500.0
        cd = free * 1.0417 + 285.0
        if self.act + ca <= self.dve + cd:
            self.act += ca
            return "act"
        self.dve += cd
        return "dve"

    def charge_dve(self, free):
        self.dve += free * 1.0417 + 285.0

    def exp(self, out, in_):
        import os
        import concourse.mybir as mybir
        if os.environ.get("K_NO_SCHRAU") or self.pick(out.free_size()) == "act":
            self.nc.scalar.activation(
                out, in_, mybir.ActivationFunctionType.Exp, scale=SCALE)
        else:
            self.nc.vector.tensor_scalar(
                out.bitcast(mybir.dt.int16), in_, SCH_A, SCH_B,
                mybir.AluOpType.mult, mybir.AluOpType.add)

    def copy(self, out, in_):
        # scheduler-chosen engine (AnyTensorCopy) beats static assignment
        self.nc.any.tensor_copy(out, in_)

    def scale(self, out, in_, r):
        import concourse.mybir as mybir
        if self.pick(out.free_size()) == "act":
            self.nc.scalar.activation(
                out, in_, mybir.ActivationFunctionType.Copy, scale=r)
        else:
            self.nc.vector.tensor_scalar(out, in_, r, None,
                                         mybir.AluOpType.mult)


def _build_nc():
    import concourse.bacc as bacc
    import concourse.bass as bass
    import concourse.masks as masks
    import concourse.mybir as mybir
    import concourse.tile as tile

    f32 = mybir.dt.float32
    f32r = mybir.dt.float32r
    bf16 = mybir.dt.bfloat16

    nc = bacc.Bacc(None, target_bir_lowering=False)

    ct = nc.dram_tensor("ct", [INPUT_DIM, M], bf16, kind="ExternalInput")   # context[b].T
    xt = nc.dram_tensor("xt", [QUERY_DIM, N], bf16, kind="ExternalInput")   # x[b].T
    wq = nc.dram_tensor("wq", [QUERY_DIM, HPC * DIM_HEAD], bf16, kind="ExternalInput")
    wk = nc.dram_tensor("wk", [INPUT_DIM, HPC * DIM_HEAD], bf16, kind="ExternalInput")
    wv = nc.dram_tensor("wv", [INPUT_DIM, HPC * DIM_HEAD], bf16, kind="ExternalInput")
    wo = nc.dram_tensor("wo", [2, 2 * DIM_HEAD, QUERY_DIM], bf16, kind="ExternalInput")
    bo2 = nc.dram_tensor("bo2", [1, QUERY_DIM], f32, kind="ExternalInput")  # bo / 2
    out = nc.dram_tensor("out", [N, QUERY_DIM], bf16, kind="ExternalOutput")

    ct_r = ct[:, :].rearrange("(t p) m -> p t m", p=128)    # [128, 2, M]
    xt_r = xt[:, :].rearrange("(t p) n -> p t n", p=128)    # [128, 4, N]
    wq_r = wq[:, :].rearrange("(t p) d -> p t d", p=128)    # [128, 4, 256]
    wk_r = wk[:, :].rearrange("(t p) d -> p t d", p=128)    # [128, 2, 256]
    wv_r = wv[:, :].rearrange("(t p) d -> p t d", p=128)    # [128, 2, 256]
    out_r = out[:, :].rearrange("(t p) f -> p t f", p=128)  # [128, 4, 512]

    bal_holder = {}

    with tile.TileContext(nc) as tc:
        with (
            tc.tile_pool(name="const", bufs=1) as cp,
            tc.tile_pool(name="ctp", bufs=2) as ctp,
            tc.tile_pool(name="ep", bufs=6) as ep,
            tc.tile_pool(name="scp", bufs=3, space="PSUM") as scp,
            tc.tile_pool(name="accp", bufs=1, space="PSUM") as accp,
        ):
            bal = Balancer(nc)
            bal_holder["bal"] = bal

            # ---- constants / weights ----
            xt_sb = cp.tile([128, 4, N], bf16)
            wq_sb = cp.tile([128, 4, HPC * DIM_HEAD], bf16)
            wk_sb = cp.tile([128, 2, HPC * DIM_HEAD], bf16)
            wv_sb = cp.tile([128, 2, HPC * DIM_HEAD], bf16)
            wo_sb = cp.tile([2 * DIM_HEAD, 2, QUERY_DIM], bf16)
            bo_sb = cp.tile([1, QUERY_DIM], f32)
            bo_bc = cp.tile([128, QUERY_DIM], f32)
            qt_sb = cp.tile([128, 2, N], f32r)               # pair p: rows h2*64+d
            kt_sb = cp.tile([128, 2, M], f32r)               # pair p, all m
            # v for all heads + ones column: [m%128, m//128, head, 64 v | 1]
            v_sb = cp.tile([128, M // 128, HPC, DIM_HEAD + 1], bf16)
            ident = cp.tile([128, 128], bf16)  # gpsimd-built identity
            norm_sb = cp.tile([128, 8, DIM_HEAD], bf16)      # per (nt,h2): [n, d]
            stack_sb = cp.tile([128, 4, 128], bf16)          # [h2*64+d, nt, n]
            out0_sb = cp.tile([128, 4, QUERY_DIM], f32)      # pass-0 proj + bias
            out_sb = cp.tile([128, 4, QUERY_DIM], bf16)
            recip_sb = cp.tile([128, 8, 1], f32)

            # prologue DMAs (ordered: qT production inputs first)
            nc.sync.dma_start(out=wq_sb[:], in_=wq_r)
            nc.scalar.dma_start(out=xt_sb[:], in_=xt_r)
            nc.sync.dma_start(out=wk_sb[:], in_=wk_r)

            # PE warm-up while prologue DMAs fly (clock ramps after ~3.4us)
            warm_sb = cp.tile([128, 64], f32)
            nc.vector.memset(warm_sb[:], 0.0)
            warm_ps = accp.tile([128, 512], f32, tag="acc0", name="warm_ps")
            for w in range(16):
                nc.tensor.matmul(
                    warm_ps[0:64, 0:64], lhsT=warm_sb[:], rhs=warm_sb[:],
                    start=True, stop=True, skip_group_check=True,
                )

            def produce_chunk(c):
                m0, mlen = CHUNKS[c]
                ct_t = ctp.tile([128, 2, MCHUNK], bf16, tag="ct", name=f"ct{c}")
                ct_dma = nc.sync.dma_start(
                    out=ct_t[:, :, 0:mlen], in_=ct_r[:, :, m0:m0 + mlen])
                if c >= 1:
                    for d in late_dmas:
                        tile.add_dep_helper(ct_dma.ins, d.ins, sync=False,
                                            reason="prologue before ct stream")

                def kt_group(p):
                    def go():
                        kt_ps = scp.tile([128, MCHUNK], f32, tag="sc",
                                         name=f"ktps{p}{c}")
                        for h in range(mlen // 512):
                            for t in range(2):
                                nc.tensor.matmul(
                                    kt_ps[:, h * 512:(h + 1) * 512],
                                    lhsT=wk_sb[:, t, p * 128:(p + 1) * 128],
                                    rhs=ct_t[:, t, h * 512:(h + 1) * 512],
                                    start=(t == 0), stop=(t == 1),
                                    skip_group_check=True,
                                )
                        bal.copy(kt_sb[:, p, m0:m0 + mlen], kt_ps[:, 0:mlen])
                    return go

                def v_group(s):
                    def go():
                        v_ps = scp.tile([128, MCHUNK], f32, tag="sc",
                                        name=f"vps{c}{s}")
                        for q in range(4):
                            mt = s * 4 + q
                            for t in range(2):
                                nc.tensor.matmul(
                                    v_ps[:, q * 256:(q + 1) * 256],
                                    lhsT=ct_t[:, t, mt * 128:(mt + 1) * 128],
                                    rhs=wv_sb[:, t, :],
                                    start=(t == 0), stop=(t == 1),
                                    skip_group_check=True,
                                )
                        base = m0 // 128 + s * 4
                        bal.copy(
                            v_sb[:, base:base + 4, :, 0:DIM_HEAD],
                            v_ps[:].rearrange("p (q h d) -> p q h d", q=4, h=HPC),
                        )
                    return go

                if mlen == 512:
                    return [kt_group(0), v_group(0), kt_group(1)]
                return [kt_group(0), v_group(0), v_group(1), kt_group(1)]

            def qk_exp(p, mi):
                sc = scp.tile([128, 1024], f32, tag="sc", name=f"sc{p}{mi}")
                ks = kt_sb[:, p, mi * 128:(mi + 1) * 128]
                nc.tensor.matmul(sc[:, 0:512], lhsT=ks[0:64, :],
                                 rhs=qt_sb[0:64, p, :], start=True, stop=True)
                nc.tensor.matmul(sc[:, 512:1024], lhsT=ks[64:128, :],
                                 rhs=qt_sb[64:128, p, :], start=True, stop=True)
                e_t = ep.tile([128, 1024], bf16, tag="e", name=f"e{p}{mi}")
                bal.exp(e_t[:], sc[:])
                return e_t

            def av(p, mi, e_t, acc):
                # Only nt==0 starts: start_tensor_calc marks the whole 2KB
                # psum bank pending-zero, so later groups' first writes
                # correctly overwrite; a start per group would re-mark the
                # bank and wipe earlier groups' mi==0 contribution.
                for h2 in range(2):
                    for nt in range(4):
                        nc.tensor.matmul(
                            acc[h2][:, nt * 128:nt * 128 + DIM_HEAD + 1],
                            lhsT=e_t[:, h2 * 512 + nt * 128:h2 * 512 + (nt + 1) * 128],
                            rhs=v_sb[:, mi, 2 * p + h2, :],
                            start=(mi == 0 and nt == 0),
                            stop=(mi == M // 128 - 1),
                            skip_group_check=True,
                        )

            def attention_tile(p, mi, acc):
                av(p, mi, qk_exp(p, mi), acc)

            def pass_tail(p, acc, per_nt=None):
                """acc[h2][:, nt*128 : nt*128+65]: cols 0-63 numerator, 64 den.
                reciprocal + per-partition scale -> [n, d] in SBUF, then PE
                transpose to [d, n] and stack for the projection."""
                tp_ps = scp.tile([128, 512], bf16, tag="sc", name=f"tp{p}")
                for nt in range(4):
                    for h2 in range(2):
                        r = recip_sb[:, 4 * h2 + nt, :]
                        nc.vector.reciprocal(
                            r, acc[h2][:, nt * 128 + DIM_HEAD:nt * 128 + DIM_HEAD + 1])
                        bal.charge_dve(1)
                        bal.scale(norm_sb[:, 2 * nt + h2, :],
                                  acc[h2][:, nt * 128:nt * 128 + DIM_HEAD], r)
                        nc.tensor.transpose(
                            tp_ps[h2 * 64:(h2 + 1) * 64, nt * 128:(nt + 1) * 128],
                            norm_sb[:, 2 * nt + h2, :], ident[:])
                    bal.copy(stack_sb[:, nt, :], tp_ps[:, nt * 128:(nt + 1) * 128])
                    if per_nt is not None:
                        per_nt(nt)

            def proj(p, nt):
                pr = scp.tile([128, QUERY_DIM], f32, tag="sc", name=f"pr{p}{nt}")
                nc.tensor.matmul(
                    pr[:], lhsT=stack_sb[:, nt, :],
                    rhs=wo_sb[:, p, :],
                    start=True, stop=True, skip_group_check=True,
                )
                if p == 0:
                    nc.vector.tensor_add(out0_sb[:, nt, :], pr[:], bo_bc[:])
                    bal.charge_dve(QUERY_DIM)
                else:
                    nc.vector.tensor_add(out_sb[:, nt, :], pr[:], out0_sb[:, nt, :])
                    bal.charge_dve(QUERY_DIM)
                    nc.sync.dma_start(out=out_r[:, nt, :], in_=out_sb[:, nt, :])

            # chunk-0 context DMA goes out right behind the qT inputs
            chunk0 = produce_chunk(0)

            # late prologue
            late_dmas = []
            late_dmas.append(nc.sync.dma_start(out=wv_sb[:], in_=wv_r))
            late_dmas.append(nc.sync.dma_start(
                out=wo_sb[:], in_=wo[:, :, :].rearrange("a p f -> p a f")))
            late_dmas.append(nc.sync.dma_start(out=bo_sb[:], in_=bo2[:, :]))
            masks.make_identity(nc, ident[:])
            # ones column of v_aug via strided broadcast-copy
            ones_col = cp.tile([128, 1], bf16)
            nc.vector.memset(ones_col[:], 1.0)
            _oc, _vdst = bass.broadcast_tensor_aps(
                ones_col[:, :], v_sb[:, :, :, DIM_HEAD].rearrange(
                    "p s h -> p (s h)")[:, None, :].rearrange("p o q -> p (o q)")
            )
            nc.vector.tensor_copy(_vdst, _oc)
            nc.gpsimd.partition_broadcast(bo_bc[:], bo_sb[0:1, :])

            # qT for both pairs: psum [128, 1024], pair p in cols p*512
            q_ps = scp.tile([128, 1024], f32, tag="sc", name="q_ps")
            for p in range(2):
                for t in range(4):
                    nc.tensor.matmul(
                        q_ps[:, p * 512:(p + 1) * 512],
                        lhsT=wq_sb[:, t, p * 128:(p + 1) * 128],
                        rhs=xt_sb[:, t, :],
                        start=(t == 0), stop=(t == 3),
                        skip_group_check=True,
                    )
            bal.copy(qt_sb[:, :, :], q_ps[:].rearrange("p (a n) -> p a n", a=2))

            # ---- pass 0 (heads 0,1), production pipelined one chunk ahead --
            acc0 = [accp.tile([128, 512], f32, tag=f"acc{h2}", name=f"a0{h2}")
                    for h2 in range(2)]
            for step in range(len(CHUNKS) + 1):
                prod = (chunk0 if step == 0 else produce_chunk(step)) \
                    if step < len(CHUNKS) else []
                if step >= 1:
                    pm0, pmlen = CHUNKS[step - 1]
                    atts = list(range(pm0 // 128, (pm0 + pmlen) // 128))
                else:
                    atts = []
                for i in range(max(2 * len(prod), len(atts))):
                    if i < len(atts):
                        attention_tile(0, atts[i], acc0)
                    if i % 2 == 0 and i // 2 < len(prod):
                        prod[i // 2]()
            # prefetch pass-1 scores/exp during the pass-0 tail drain
            prefetch = {mi: qk_exp(1, mi) for mi in range(5)}
            pass_tail(0, acc0)

            # ---- pass 1 (heads 2,3): pure attention from resident kT/v ----
            acc1 = [accp.tile([128, 512], f32, tag=f"acc{h2}", name=f"a1{h2}")
                    for h2 in range(2)]
            for mi in range(M // 128):
                if mi in prefetch:
                    av(1, mi, prefetch.pop(mi), acc1)
                else:
                    attention_tile(1, mi, acc1)
                if mi == 8:
                    for nt in range(4):
                        proj(0, nt)
            pass_tail(1, acc1, per_nt=lambda nt: proj(1, nt))

    nc.compile()
    return nc


def _get_nc():
    if "nc" not in _CACHE:
        _CACHE["nc"] = _build_nc()
    return _CACHE["nc"]


def _make_in_maps(x, context, Wq, Wkv, Wo, bo):
    x = np.asarray(x, dtype=np.float32)
    context = np.asarray(context, dtype=np.float32)
    Wq = np.asarray(Wq, dtype=np.float32)
    Wkv = np.asarray(Wkv, dtype=np.float32)
    Wo = np.asarray(Wo, dtype=np.float32)
    bo = np.asarray(bo, dtype=np.float32)

    Wk = Wkv[:, :ATT_DIM]
    Wv = Wkv[:, ATT_DIM:]
    bo2 = np.ascontiguousarray((bo / 2.0)[None, :])

    import ml_dtypes
    in_maps = []
    for c in range(N_CORES):
        b, g = divmod(c, 2)
        hs = g * HPC * DIM_HEAD           # column offset of this core's heads
        he = hs + HPC * DIM_HEAD
        wo_core = np.stack([
            Wo[hs + p * 128:hs + (p + 1) * 128, :] for p in range(2)
        ]).astype(ml_dtypes.bfloat16)
        in_maps.append({
            "ct": np.ascontiguousarray(context[b].T).astype(ml_dtypes.bfloat16),
            "xt": np.ascontiguousarray(x[b].T).astype(ml_dtypes.bfloat16),
            "wq": np.ascontiguousarray(Wq[:, hs:he]).astype(ml_dtypes.bfloat16),
            "wk": np.ascontiguousarray(Wk[:, hs:he]).astype(ml_dtypes.bfloat16),
            "wv": np.ascontiguousarray(Wv[:, hs:he]).astype(ml_dtypes.bfloat16),
            "wo": np.ascontiguousarray(wo_core),
            "bo2": bo2,
        })
    return in_maps


def run(inputs, trace=False, **spmd_kwargs):
    """Run the kernel; returns (full_output [B,N,QUERY_DIM], BassKernelResults)."""
    from concourse.bass_utils import run_bass_kernel_spmd

    nc = _get_nc()
    in_maps = _make_in_maps(**inputs)
    res = run_bass_kernel_spmd(
        nc, in_maps, core_ids=list(range(N_CORES)), trace=trace, **spmd_kwargs
    )
    outs = [np.asarray(r["out"], dtype=np.float32) for r in res.results]
    full = np.empty((B, N, QUERY_DIM), dtype=np.float32)
    for b in range(B):
        full[b] = outs[2 * b] + outs[2 * b + 1]
    return full, res


def kernel(**inputs) -> np.ndarray:
    full, _ = run(inputs, trace=False)
    return full
